# revision 11
# baseline (speedup 1.0000x reference)
"""Trainium2 Bass kernel for windowed 3D attention (sparse_attention).

Per window (256 windows on a 16x16 grid): N=294 tokens, d=256, 8 heads x 32.
qkv = x @ w_qkv.T ; A = softmax(q k^T/sqrt(dh) + bias) ; out = (A v) @ w_out.T
Sharding: data-parallel over the grid; core s takes X-rows [2s, 2s+2) = 32 windows.

v5 design (v2: ~469us, v3: ~429us):
  - bias applied as A = exp(S) * expB (expB host-precomputed, bf16 const);
    quad multiplies: jc0/jc1 on DVE (all-bf16 SBUF step-1 => 2x perf mode,
    ~770ns vs GPSIMD ~2400ns), jc2 on GPSIMD for queue balance.
  - S matmuls in FP8 (e4m3) with perf_mode=DoubleRow: Q/K stored as fp8
    [128, {q0,q1,k0,k1}, 2, 304] where pair-block 0 holds the data and the
    K-side pair-block 1 is zeroed (GPSIMD memset) — DoubleRow contracts
    sum_i W[:,i].T @ X[:,i], so zero K-odd blocks keep the math exact while
    the moving stream runs at 2 cols/cycle (S stream halves).  304 = 294
    padded so the pair-block stride is 16-aligned.  Host splits the softmax
    scale sqrt-evenly into Wq and Wk so both land mid-range in fp8.
  - rowsums via ones[jn,32] matmuls: row-sum REPLICATED across each head's
    32 partitions ([he, i] layout) -> reciprocal_approx_fast -> normalize O^T
    directly from PSUM on DVE.  Y^T = wout^T.T @ O_norm^T d-major.
  - S^T tiles [128, 2, 512] fp32 (2 banks, one head per bank).  HARD HW
    CONSTRAINT: a matmul PSUM output must start at a PSUM bank boundary
    (CoreSim does NOT model this).  Also: engine ops may not access >32
    partitions from a non-aligned partition start (BIR verifier).
  - S matmuls single-shot, K=32 row-packed 4-way via tile_position
    (32*(h%4), 0); RS/AV K=jn col-packed 4-way at (0, 32*hh).
  - PSUM: S-ring 3 x 2 banks + misc ring 2 x 1 bank = 8 banks.
  - 2-window QKV runway: prologue runs qkv(0)+qkv(1); in-loop fillers are
    qkv(w+2) + tail(w-1) interleaved with window w's 12 S/exp slots.
"""

import os
from contextlib import ExitStack

import numpy as np
import ml_dtypes

import concourse.bass as bass
import concourse.mybir as mybir
import concourse.tile as tile
from concourse import bacc
from concourse.bass_utils import run_bass_kernel_spmd

F32 = mybir.dt.float32
BF16 = mybir.dt.bfloat16
FP8 = mybir.dt.float8e4

L, W, D, H = 6, 7, 256, 8
DH = D // H                      # 32
N = L * W * W                    # 294
NP = 304                         # N padded to 16-elem multiple (DoubleRow stride)
GX = GY = 16
NCORES = 8
XPC = GX // NCORES               # X-rows per core
NW = int(os.environ.get("KNW", XPC * GY))   # 32 windows per core (overridable for debug)
TOK = NW * N                     # 9408 tokens per core
SCALE = DH ** -0.5
KFP8 = bool(int(os.environ.get("KFP8", "1")))   # fp8 DoubleRow S-stage

CH = [(0, 128), (128, 128), (256, 38)]    # j / i chunks


TRACE = False     # set by test.py for profiling runs
_CACHE = {}


def _body(ctx, tc, xT, wqkvT, woutT, expBT, y):
    nc = tc.nc

    const = ctx.enter_context(tc.tile_pool(name="const", bufs=1))
    xpool = ctx.enter_context(tc.tile_pool(name="xin", bufs=4))
    qkpool = ctx.enter_context(tc.tile_pool(name="qk", bufs=4))
    vpool = ctx.enter_context(tc.tile_pool(name="vtok", bufs=4))
    arawpool = ctx.enter_context(tc.tile_pool(name="araw", bufs=6))
    atpool = ctx.enter_context(tc.tile_pool(name="at", bufs=4))
    rrpool = ctx.enter_context(tc.tile_pool(name="rr", bufs=3))
    onpool = ctx.enter_context(tc.tile_pool(name="onorm", bufs=3))
    ypool = ctx.enter_context(tc.tile_pool(name="ysb", bufs=3))

    # PSUM: S-ring 3 x [128,2,512]f32 (2 banks each) + misc ring 2 x 1 bank
    ps_s = ctx.enter_context(tc.tile_pool(name="ps_s", bufs=3, space="PSUM"))
    ps_m = ctx.enter_context(tc.tile_pool(name="ps_m", bufs=2, space="PSUM"))

    # ---- resident constants ----
    # Declare all const tiles, but order the DMA queue so the first window's
    # dependencies (wqkv, then x0 — emitted by the caller right after
    # wqkv_dma()) land first; wv/wout follow, the 1.8MB expb goes last.
    wqkv_s = const.tile([128, 2, 2 * D], BF16)     # Q^T,K^T weight cols (pre-scaled)
    wv_s = const.tile([128, 2, D], BF16)
    wout_s = const.tile([128, 2, D], BF16)         # w_out^T [he, dout] he-chunked
    expb_s = const.tile([128, 3, H * N], BF16)     # exp(B^T)[j, (h,i)], j-chunked
    ones_b = const.tile([128, 32], BF16)
    nc.vector.memset(ones_b, 1.0)

    def wqkv_dma():
        nc.sync.dma_start(out=wqkv_s, in_=wqkvT.rearrange("(c p) n -> p c n", c=2)[:, :, 0:2 * D])

    def wtail_dma():
        nc.sync.dma_start(out=wv_s, in_=wqkvT.rearrange("(c p) n -> p c n", c=2)[:, :, 2 * D:3 * D])
        nc.sync.dma_start(out=wout_s, in_=woutT.rearrange("(c p) n -> p c n", c=2))

    def load_expb():
        # deferred: 1.8MB const queued AFTER the early x DMAs so the first
        # windows' QKV isn't stuck behind it on the DMA queue
        for jc, (j0, jn) in enumerate(CH):
            nc.sync.dma_start(out=expb_s[:jn, jc, :], in_=expBT[j0:j0 + jn, :])

    state = {}   # per-window live tiles

    def load_x(w):
        t0 = w * N
        xw = xpool.tile([128, 2, N], BF16, tag="xw", name=f"xw{w}")
        nc.sync.dma_start(out=xw, in_=xT.rearrange("(c p) t -> p c t", c=2)[:, :, t0:t0 + N])
        state[("x", w)] = xw

    def qkv_pieces(w):
        """7 closures: QKV projection of window w, one psum tile each."""
        xw = state[("x", w)]
        if KFP8:
            # fp8 DoubleRow layout: [m, pair, NP]; ALL pair-1 blocks zeroed —
            # uninitialized fp8 bytes can be NaN, and NaN*0 = NaN even against
            # the zeroed K side, so the Q-side junk must be cleared too.
            qk_sb = qkpool.tile([128, 4, 2, NP], FP8, tag="qk", name=f"qk{w}")
            nc.gpsimd.memset(qk_sb[:, :, 1, :], 0.0)
        else:
            qk_sb = qkpool.tile([128, 4, N], BF16, tag="qk", name=f"qk{w}")
        vtok = vpool.tile([128, 3, D], BF16, tag="vt", name=f"vt{w}")
        state[("qk", w)] = qk_sb
        state[("v", w)] = vtok

        def qk_piece(m):
            def run():
                pq = ps_m.tile([128, 512], F32, tag="ring", name=f"pq{w}_{m}")
                for kc in range(2):
                    nc.tensor.matmul(
                        pq[:, :N], wqkv_s[:, kc, m * 128:(m + 1) * 128], xw[:, kc, :],
                        start=(kc == 0), stop=(kc == 1))
                if KFP8:
                    nc.vector.tensor_copy(qk_sb[:, m, 0, :N], pq[:, :N])
                else:
                    nc.vector.tensor_copy(qk_sb[:, m, :], pq[:, :N])
            return run

        def v_piece(jc):
            def run():
                j0, jn = CH[jc]
                pv = ps_m.tile([128, 512], F32, tag="ring", name=f"pv{w}_{jc}")
                for kc in range(2):
                    nc.tensor.matmul(
                        pv[:jn, :D], xw[:, kc, j0:j0 + jn], wv_s[:, kc, :],
                        start=(kc == 0), stop=(kc == 1))
                nc.vector.tensor_copy(vtok[:jn, jc, :], pv[:jn, :D])
            return run

        return [qk_piece(m) for m in range(4)] + [v_piece(jc) for jc in range(3)]

    def tail_pieces(w):
        """6 closures: rowsums+recip / AV+normalize / out-proj of window w."""
        at = state[("at", w)]
        vtok = state[("v", w)]
        rr = rrpool.tile([128, 2, N], F32, tag="rr", name=f"rr{w}")
        onrm = onpool.tile([128, 2, N], BF16, tag="onrm", name=f"on{w}")
        ysb = ypool.tile([128, 2, N], F32, tag="ysb", name=f"ysb{w}")
        t0 = w * N

        def rs_piece(g):
            def run():
                prs = ps_m.tile([128, 512], F32, tag="ring", name=f"prs{w}_{g}")
                for jc, (j0, jn) in enumerate(CH):
                    for hh in range(4):
                        h = 4 * g + hh
                        nc.tensor.matmul(
                            prs[32 * hh:32 * hh + 32, :N],
                            ones_b[:jn, :], at[:jn, jc, h, :],
                            start=(jc == 0), stop=(jc == 2),
                            tile_position=(0, 32 * hh), skip_group_check=True)
                nc.vector.reciprocal_approx_fast(out=rr[:, g, :], in_=prs[:, :N])
            return run

        def av_piece(g):
            def run():
                po = ps_m.tile([128, 512], F32, tag="ring", name=f"po{w}_{g}")
                for jc, (j0, jn) in enumerate(CH):
                    for hh in range(4):
                        h = 4 * g + hh
                        nc.tensor.matmul(
                            po[32 * hh:32 * hh + 32, :N],
                            vtok[:jn, jc, 32 * h:32 * h + 32], at[:jn, jc, h, :],
                            start=(jc == 0), stop=(jc == 2),
                            tile_position=(0, 32 * hh), skip_group_check=True)
                nc.vector.tensor_tensor(
                    out=onrm[:, g, :], in0=po[:, :N], in1=rr[:, g, :],
                    op=mybir.AluOpType.mult)
            return run

        def y_piece(d):
            def run():
                py = ps_m.tile([128, 512], F32, tag="ring", name=f"py{w}_{d}")
                for g in range(2):
                    nc.tensor.matmul(
                        py[:, :N], wout_s[:, g, 128 * d:128 * (d + 1)], onrm[:, g, :],
                        start=(g == 0), stop=(g == 1))
                nc.vector.tensor_copy(ysb[:, d, :], py[:, :N])
                nc.sync.dma_start(out=y[128 * d:128 * (d + 1), t0:t0 + N], in_=ysb[:, d, :])
            return run

        return [rs_piece(0), rs_piece(1), av_piece(0), av_piece(1),
                y_piece(0), y_piece(1)]

    def s_slot(w, jc, g2, qd_tiles):
        """S matmuls + exp for heads (2*g2, 2*g2+1) of (window w, j-chunk jc),
        then (after the second tile of a quad) the bias multiply."""
        qk_sb = state[("qk", w)]
        at = state[("at", w)]
        j0, jn = CH[jc]
        s_t = ps_s.tile([128, 2, 512], F32, tag="sring", name=f"s{w}_{jc}_{g2}")
        for p in range(2):
            h = 2 * g2 + p
            hh = h % 4
            g = h // 4
            if KFP8:
                nc.tensor.matmul(
                    s_t[:jn, p, :N],
                    qk_sb[32 * hh:32 * hh + 32, 2 + g, :, j0:j0 + jn],  # K^T [32,2,jn]
                    qk_sb[32 * hh:32 * hh + 32, g, :, :N],             # Q^T [32,2,294]
                    start=True, stop=True, skip_group_check=True,
                    tile_position=(32 * hh, 0),
                    perf_mode=mybir.MatmulPerfMode.DoubleRow)
            else:
                nc.tensor.matmul(
                    s_t[:jn, p, :N],
                    qk_sb[32 * hh:32 * hh + 32, 2 + g, j0:j0 + jn],    # K^T [32, jn]
                    qk_sb[32 * hh:32 * hh + 32, g, :],                 # Q^T [32, 294]
                    start=True, stop=True, skip_group_check=True,
                    tile_position=(32 * hh, 0))
        # quad = two consecutive 2-head tiles share one araw tile so the
        # bias multiply runs as one [jn, 4, 294] op
        qd = g2 // 2
        sub = g2 % 2
        if sub == 0:
            qd_tiles[qd] = arawpool.tile(
                [128, 4, N], BF16, tag="araw", name=f"ar{w}_{jc}_{qd}")
        araw = qd_tiles[qd]
        nc.scalar.activation(
            araw[:jn, 2 * sub:2 * sub + 2, :], s_t[:jn, :, :N],
            mybir.ActivationFunctionType.Exp)
        if sub == 1:
            # bias multiply for the whole quad (heads 4*qd .. 4*qd+4).
            # jc0/jc1 on DVE (2x perf mode ~770ns); jc2 on GPSIMD (~2400ns)
            # to keep the DVE queue short.
            eng = nc.vector if jc < 2 else nc.gpsimd
            eng.tensor_tensor(
                out=at[:jn, jc, 4 * qd:4 * qd + 4, :],
                in0=araw[:jn, :, :],
                in1=expb_s[:jn, jc, 4 * qd * N:(4 * qd + 4) * N].rearrange(
                    "p (h i) -> p h i", h=4),
                op=mybir.AluOpType.mult)

    # ---- prologue: 2-window QKV runway so in-loop fillers are w+2 ----
    wqkv_dma()
    load_x(0)
    load_x(1)
    wtail_dma()
    load_x(2)
    load_expb()
    load_x(3)
    for f in qkv_pieces(0):
        f()
    for f in qkv_pieces(1):
        f()

    # ---- pipelined window loop ----
    for w in range(NW):
        if w + 4 < NW:
            load_x(w + 4)
        state[("at", w)] = atpool.tile([128, 3, H, N], BF16, tag="at", name=f"at{w}")

        # tail(w-1) first: its DVE recip/mult chain gates ps_m ring reuse
        # (tight); qkv(w+2) feeds S-slots two windows out (loose).
        fillers = []
        if w > 0:
            fillers += tail_pieces(w - 1)
        if w + 2 < NW:
            fillers += qkv_pieces(w + 2)

        # interleave ~1 filler piece per S slot to keep PE fed without
        # delaying the next S tile behind a long PE burst
        nslots = 12
        total = len(fillers)
        taken = 0
        for si, (jc, g2) in enumerate([(j, g) for j in range(3) for g in range(4)]):
            if g2 == 0:
                qd_tiles = {}
            s_slot(w, jc, g2, qd_tiles)
            want = total * (si + 1) // nslots
            while taken < want:
                fillers[taken]()
                taken += 1
        while taken < total:
            fillers[taken]()
            taken += 1

    # final tail
    for f in tail_pieces(NW - 1):
        f()


def _build():
    if "nc" in _CACHE:
        return _CACHE["nc"]
    nc = bacc.Bacc("TRN2", target_bir_lowering=False)
    xT = nc.dram_tensor("xT", [D, TOK], BF16, kind="ExternalInput").ap()
    wqkvT = nc.dram_tensor("wqkvT", [D, 3 * D], BF16, kind="ExternalInput").ap()
    woutT = nc.dram_tensor("woutT", [D, D], BF16, kind="ExternalInput").ap()
    expBT = nc.dram_tensor("expBT", [N, H * N], BF16, kind="ExternalInput").ap()
    y = nc.dram_tensor("y", [D, TOK], F32, kind="ExternalOutput").ap()
    with tile.TileContext(nc) as tc, ExitStack() as ctx:
        _body(ctx, tc, xT, wqkvT, woutT, expBT, y)
    nc.compile()
    _CACHE["nc"] = nc
    return nc


def kernel(x, w_qkv, w_out, bias_table, rel_idx):
    x = np.asarray(x, dtype=np.float32)
    w_qkv = np.asarray(w_qkv, dtype=np.float32)
    w_out = np.asarray(w_out, dtype=np.float32)
    bias_table = np.asarray(bias_table, dtype=np.float32)
    rel_idx = np.asarray(rel_idx)

    # host-side layout prep
    # x[0]: [l, X, Y, w1, w2, d] -> xT [d, (X Y l w1 w2)] bf16
    xt = np.ascontiguousarray(
        x[0].transpose(5, 1, 2, 0, 3, 4)).reshape(D, GX * GY * N).astype(ml_dtypes.bfloat16)
    wq = w_qkv.copy()
    if KFP8:
        # split the attention scale sqrt-evenly so q and k land mid-range in fp8
        s = SCALE ** 0.5
        wq[:D] *= s
        wq[D:2 * D] *= s
    else:
        wq[:D] *= SCALE                    # fold attention scale into Wq
    wqkvT = np.ascontiguousarray(wq.T).astype(ml_dtypes.bfloat16)
    woutT = np.ascontiguousarray(w_out.T).astype(ml_dtypes.bfloat16)
    bias = bias_table[rel_idx]             # [i, j, h]
    expBT = np.ascontiguousarray(
        np.exp(bias.transpose(1, 2, 0))).reshape(N, H * N).astype(ml_dtypes.bfloat16)

    nc = _build()
    in_maps = []
    for s in range(NCORES):
        xs = np.ascontiguousarray(xt[:, s * TOK:(s + 1) * TOK])
        in_maps.append({"xT": xs, "wqkvT": wqkvT, "woutT": woutT, "expBT": expBT})

    res = run_bass_kernel_spmd(nc, in_maps, core_ids=list(range(NCORES)), trace=TRACE)
    _CACHE["res"] = res
    if TRACE and res.exec_time_ns is not None:
        print(f"HW exec time: {res.exec_time_ns} ns")
        _CACHE["exec_time_ns"] = res.exec_time_ns

    # gather: per-core y [256, 9408] d-major -> [1, l, X, Y, w1, w2, d]
    out = np.empty((1, L, GX, GY, W, W, D), dtype=np.float32)
    for s in range(NCORES):
        yc = res.results[s]["y"].reshape(D, XPC, GY, L, W, W)
        out[0, :, s * XPC:(s + 1) * XPC] = yc.transpose(3, 1, 2, 4, 5, 0)
    return out


# revision 13
# speedup vs baseline: 1.0794x; 1.0794x over previous
"""Trainium2 Bass kernel for windowed 3D attention (sparse_attention).

Per window (256 windows on a 16x16 grid): N=294 tokens, d=256, 8 heads x 32.
qkv = x @ w_qkv.T ; A = softmax(q k^T/sqrt(dh) + bias) ; out = (A v) @ w_out.T
Sharding: data-parallel over the grid; core s takes X-rows [2s, 2s+2) = 32 windows.

v5 design (v2: ~469us, v3: ~429us):
  - bias applied as A = exp(S) * expB (expB host-precomputed, bf16 const);
    quad multiplies: jc0/jc1 on DVE (all-bf16 SBUF step-1 => 2x perf mode,
    ~770ns vs GPSIMD ~2400ns), jc2 on GPSIMD for queue balance.
  - S matmuls in FP8 (e4m3) with perf_mode=DoubleRow: Q/K stored as fp8
    [128, {q0,q1,k0,k1}, 2, 304] where pair-block 0 holds the data and the
    K-side pair-block 1 is zeroed (GPSIMD memset) — DoubleRow contracts
    sum_i W[:,i].T @ X[:,i], so zero K-odd blocks keep the math exact while
    the moving stream runs at 2 cols/cycle (S stream halves).  304 = 294
    padded so the pair-block stride is 16-aligned.  Host splits the softmax
    scale sqrt-evenly into Wq and Wk so both land mid-range in fp8.
  - rowsums via ones[jn,32] matmuls: row-sum REPLICATED across each head's
    32 partitions ([he, i] layout) -> reciprocal_approx_fast -> normalize O^T
    directly from PSUM on DVE.  Y^T = wout^T.T @ O_norm^T d-major.
  - S^T tiles [128, 2, 512] fp32 (2 banks, one head per bank).  HARD HW
    CONSTRAINT: a matmul PSUM output must start at a PSUM bank boundary
    (CoreSim does NOT model this).  Also: engine ops may not access >32
    partitions from a non-aligned partition start (BIR verifier).
  - S matmuls single-shot, K=32 row-packed 4-way via tile_position
    (32*(h%4), 0); RS/AV K=jn col-packed 4-way at (0, 32*hh).
  - PSUM: S-ring 3 x 2 banks + misc ring 2 x 1 bank = 8 banks.
  - 2-window QKV runway: prologue runs qkv(0)+qkv(1); in-loop fillers are
    qkv(w+2) + tail(w-1) interleaved with window w's 12 S/exp slots.
"""

import os
from contextlib import ExitStack

import numpy as np
import ml_dtypes

import concourse.bass as bass
import concourse.mybir as mybir
import concourse.tile as tile
from concourse import bacc
from concourse.bass_utils import run_bass_kernel_spmd

F32 = mybir.dt.float32
BF16 = mybir.dt.bfloat16
FP8 = mybir.dt.float8e4

L, W, D, H = 6, 7, 256, 8
DH = D // H                      # 32
N = L * W * W                    # 294
NP = 304                         # N padded to 16-elem multiple (DoubleRow stride)
GX = GY = 16
NCORES = 8
XPC = GX // NCORES               # X-rows per core
NW = int(os.environ.get("KNW", XPC * GY))   # 32 windows per core (overridable for debug)
TOK = NW * N                     # 9408 tokens per core
SCALE = DH ** -0.5
KFP8 = bool(int(os.environ.get("KFP8", "1")))   # fp8 DoubleRow S-stage

CH = [(0, 128), (128, 128), (256, 38)]    # j / i chunks


TRACE = False     # set by test.py for profiling runs
_CACHE = {}


def _body(ctx, tc, xT, wqkvT, woutT, expBT, y):
    nc = tc.nc

    const = ctx.enter_context(tc.tile_pool(name="const", bufs=1))
    xpool = ctx.enter_context(tc.tile_pool(name="xin", bufs=4))
    qkpool = ctx.enter_context(tc.tile_pool(name="qk", bufs=4))
    vpool = ctx.enter_context(tc.tile_pool(name="vtok", bufs=4))
    arawpool = ctx.enter_context(tc.tile_pool(name="araw", bufs=6))
    atpool = ctx.enter_context(tc.tile_pool(name="at", bufs=4))
    rrpool = ctx.enter_context(tc.tile_pool(name="rr", bufs=3))
    onpool = ctx.enter_context(tc.tile_pool(name="onorm", bufs=3))
    ypool = ctx.enter_context(tc.tile_pool(name="ysb", bufs=3))

    # PSUM: S-ring 3 x [128,2,512]f32 (2 banks each) + misc ring 2 x 1 bank
    ps_s = ctx.enter_context(tc.tile_pool(name="ps_s", bufs=3, space="PSUM"))
    ps_m = ctx.enter_context(tc.tile_pool(name="ps_m", bufs=2, space="PSUM"))

    # ---- resident constants ----
    # Declare all const tiles, but order the DMA queue so the first window's
    # dependencies (wqkv, then x0 — emitted by the caller right after
    # wqkv_dma()) land first; wv/wout follow, the 1.8MB expb goes last.
    wqkv_s = const.tile([128, 2, 2 * D], BF16)     # Q^T,K^T weight cols (pre-scaled)
    wv_s = const.tile([128, 2, D], BF16)
    wout_s = const.tile([128, 2, D], BF16)         # w_out^T [he, dout] he-chunked
    expb_s = const.tile([128, 3, H * N], BF16)     # exp(B^T)[j, (h,i)], j-chunked
    ones_b = const.tile([128, 32], BF16)
    nc.vector.memset(ones_b, 1.0)

    def wqkv_dma():
        nc.sync.dma_start(out=wqkv_s, in_=wqkvT.rearrange("(c p) n -> p c n", c=2)[:, :, 0:2 * D])

    def wtail_dma():
        nc.sync.dma_start(out=wv_s, in_=wqkvT.rearrange("(c p) n -> p c n", c=2)[:, :, 2 * D:3 * D])
        nc.sync.dma_start(out=wout_s, in_=woutT.rearrange("(c p) n -> p c n", c=2))

    def load_expb():
        # deferred: 1.8MB const queued AFTER the early x DMAs so the first
        # windows' QKV isn't stuck behind it on the DMA queue
        for jc, (j0, jn) in enumerate(CH):
            nc.sync.dma_start(out=expb_s[:jn, jc, :], in_=expBT[j0:j0 + jn, :])

    state = {}   # per-window live tiles

    def load_x(w):
        t0 = w * N
        xw = xpool.tile([128, 2, N], BF16, tag="xw", name=f"xw{w}")
        nc.sync.dma_start(out=xw, in_=xT.rearrange("(c p) t -> p c t", c=2)[:, :, t0:t0 + N])
        state[("x", w)] = xw

    def qkv_pieces(w):
        """7 closures: QKV projection of window w, one psum tile each."""
        xw = state[("x", w)]
        if KFP8:
            # fp8 DoubleRow layout: [m, pair, NP]; ALL pair-1 blocks zeroed —
            # uninitialized fp8 bytes can be NaN, and NaN*0 = NaN even against
            # the zeroed K side, so the Q-side junk must be cleared too.
            qk_sb = qkpool.tile([128, 4, 2, NP], FP8, tag="qk", name=f"qk{w}")
            nc.gpsimd.memset(qk_sb[:, :, 1, :], 0.0)
        else:
            qk_sb = qkpool.tile([128, 4, N], BF16, tag="qk", name=f"qk{w}")
        vtok = vpool.tile([128, 3, D], BF16, tag="vt", name=f"vt{w}")
        state[("qk", w)] = qk_sb
        state[("v", w)] = vtok

        def qk_piece(m):
            def run():
                pq = ps_m.tile([128, 512], F32, tag="ring", name=f"pq{w}_{m}")
                for kc in range(2):
                    nc.tensor.matmul(
                        pq[:, :N], wqkv_s[:, kc, m * 128:(m + 1) * 128], xw[:, kc, :],
                        start=(kc == 0), stop=(kc == 1))
                if KFP8:
                    nc.vector.tensor_copy(qk_sb[:, m, 0, :N], pq[:, :N])
                else:
                    nc.vector.tensor_copy(qk_sb[:, m, :], pq[:, :N])
            return run

        def v_piece(jc):
            def run():
                j0, jn = CH[jc]
                pv = ps_m.tile([128, 512], F32, tag="ring", name=f"pv{w}_{jc}")
                for kc in range(2):
                    nc.tensor.matmul(
                        pv[:jn, :D], xw[:, kc, j0:j0 + jn], wv_s[:, kc, :],
                        start=(kc == 0), stop=(kc == 1))
                nc.vector.tensor_copy(vtok[:jn, jc, :], pv[:jn, :D])
            return run

        return [qk_piece(m) for m in range(4)] + [v_piece(jc) for jc in range(3)]

    def tail_pieces(w):
        """6 closures: rowsums+recip / AV+normalize / out-proj of window w."""
        at = state[("at", w)]
        vtok = state[("v", w)]
        rr = rrpool.tile([128, 2, N], F32, tag="rr", name=f"rr{w}")
        onrm = onpool.tile([128, 2, N], BF16, tag="onrm", name=f"on{w}")
        ysb = ypool.tile([128, 2, N], F32, tag="ysb", name=f"ysb{w}")
        t0 = w * N

        def rs_piece(g):
            def run():
                prs = ps_m.tile([128, 512], F32, tag="ring", name=f"prs{w}_{g}")
                for jc, (j0, jn) in enumerate(CH):
                    for hh in range(4):
                        h = 4 * g + hh
                        nc.tensor.matmul(
                            prs[32 * hh:32 * hh + 32, :N],
                            ones_b[:jn, :], at[:jn, jc, h, :],
                            start=(jc == 0), stop=(jc == 2),
                            tile_position=(0, 32 * hh), skip_group_check=True)
                nc.vector.reciprocal_approx_fast(out=rr[:, g, :], in_=prs[:, :N])
            return run

        def av_piece(g):
            def run():
                po = ps_m.tile([128, 512], F32, tag="ring", name=f"po{w}_{g}")
                for jc, (j0, jn) in enumerate(CH):
                    for hh in range(4):
                        h = 4 * g + hh
                        nc.tensor.matmul(
                            po[32 * hh:32 * hh + 32, :N],
                            vtok[:jn, jc, 32 * h:32 * h + 32], at[:jn, jc, h, :],
                            start=(jc == 0), stop=(jc == 2),
                            tile_position=(0, 32 * hh), skip_group_check=True)
                nc.vector.tensor_tensor(
                    out=onrm[:, g, :], in0=po[:, :N], in1=rr[:, g, :],
                    op=mybir.AluOpType.mult)
            return run

        def y_piece(d):
            def run():
                py = ps_m.tile([128, 512], F32, tag="ring", name=f"py{w}_{d}")
                for g in range(2):
                    nc.tensor.matmul(
                        py[:, :N], wout_s[:, g, 128 * d:128 * (d + 1)], onrm[:, g, :],
                        start=(g == 0), stop=(g == 1))
                nc.vector.tensor_copy(ysb[:, d, :], py[:, :N])
                nc.sync.dma_start(out=y[128 * d:128 * (d + 1), t0:t0 + N], in_=ysb[:, d, :])
            return run

        return [rs_piece(0), rs_piece(1), av_piece(0), av_piece(1),
                y_piece(0), y_piece(1)]

    def s_slot(w, jc, g2, qd_tiles):
        """S matmuls + exp for heads (2*g2, 2*g2+1) of (window w, j-chunk jc),
        then (after the second tile of a quad) the bias multiply."""
        qk_sb = state[("qk", w)]
        at = state[("at", w)]
        j0, jn = CH[jc]
        s_t = ps_s.tile([128, 2, 512], F32, tag="sring", name=f"s{w}_{jc}_{g2}")
        for p in range(2):
            h = 2 * g2 + p
            hh = h % 4
            g = h // 4
            if KFP8:
                nc.tensor.matmul(
                    s_t[:jn, p, :N],
                    qk_sb[32 * hh:32 * hh + 32, 2 + g, :, j0:j0 + jn],  # K^T [32,2,jn]
                    qk_sb[32 * hh:32 * hh + 32, g, :, :N],             # Q^T [32,2,294]
                    start=True, stop=True, skip_group_check=True,
                    tile_position=(32 * hh, 0),
                    perf_mode=mybir.MatmulPerfMode.DoubleRow)
            else:
                nc.tensor.matmul(
                    s_t[:jn, p, :N],
                    qk_sb[32 * hh:32 * hh + 32, 2 + g, j0:j0 + jn],    # K^T [32, jn]
                    qk_sb[32 * hh:32 * hh + 32, g, :],                 # Q^T [32, 294]
                    start=True, stop=True, skip_group_check=True,
                    tile_position=(32 * hh, 0))
        # quad = two consecutive 2-head tiles share one araw tile so the
        # bias multiply runs as one [jn, 4, 294] op
        qd = g2 // 2
        sub = g2 % 2
        if sub == 0:
            qd_tiles[qd] = arawpool.tile(
                [128, 4, N], BF16, tag="araw", name=f"ar{w}_{jc}_{qd}")
        araw = qd_tiles[qd]
        nc.scalar.activation(
            araw[:jn, 2 * sub:2 * sub + 2, :], s_t[:jn, :, :N],
            mybir.ActivationFunctionType.Exp)
        if sub == 1:
            # bias multiply for the whole quad (heads 4*qd .. 4*qd+4).
            # jc0/jc1 on DVE (2x perf mode ~770ns); jc2 on GPSIMD (~2400ns)
            # to keep the DVE queue short.
            eng = nc.vector if jc < 2 else nc.gpsimd
            eng.tensor_tensor(
                out=at[:jn, jc, 4 * qd:4 * qd + 4, :],
                in0=araw[:jn, :, :],
                in1=expb_s[:jn, jc, 4 * qd * N:(4 * qd + 4) * N].rearrange(
                    "p (h i) -> p h i", h=4),
                op=mybir.AluOpType.mult)

    # ---- prologue: 2-window QKV runway so in-loop fillers are w+2 ----
    wqkv_dma()
    load_x(0)
    load_x(1)
    wtail_dma()
    load_x(2)
    load_expb()
    load_x(3)
    for f in qkv_pieces(0):
        f()
    for f in qkv_pieces(1):
        f()

    # ---- pipelined window loop ----
    for w in range(NW):
        if w + 4 < NW:
            load_x(w + 4)
        state[("at", w)] = atpool.tile([128, 3, H, N], BF16, tag="at", name=f"at{w}")

        # qkv(w+2) first: tail(w-1) pieces early in the window stall the
        # in-order PE queue on at(w-1) readiness (measured +110us when
        # tail-first).
        fillers = []
        if w + 2 < NW:
            fillers += qkv_pieces(w + 2)
        if w > 0:
            fillers += tail_pieces(w - 1)

        # interleave ~1 filler piece per S slot to keep PE fed without
        # delaying the next S tile behind a long PE burst
        nslots = 12
        total = len(fillers)
        taken = 0
        # jc2 slots first: their bias-mult quads ride GPSIMD (~2.4us each);
        # firing them early gives a full window of slack before tail(w)
        # matmuls consume at[jc2] from the in-order PE queue.
        for si, (jc, g2) in enumerate([(j, g) for j in (2, 0, 1) for g in range(4)]):
            if g2 == 0:
                qd_tiles = {}
            s_slot(w, jc, g2, qd_tiles)
            want = total * (si + 1) // nslots
            while taken < want:
                fillers[taken]()
                taken += 1
        while taken < total:
            fillers[taken]()
            taken += 1

    # final tail
    for f in tail_pieces(NW - 1):
        f()


def _build():
    if "nc" in _CACHE:
        return _CACHE["nc"]
    nc = bacc.Bacc("TRN2", target_bir_lowering=False)
    xT = nc.dram_tensor("xT", [D, TOK], BF16, kind="ExternalInput").ap()
    wqkvT = nc.dram_tensor("wqkvT", [D, 3 * D], BF16, kind="ExternalInput").ap()
    woutT = nc.dram_tensor("woutT", [D, D], BF16, kind="ExternalInput").ap()
    expBT = nc.dram_tensor("expBT", [N, H * N], BF16, kind="ExternalInput").ap()
    y = nc.dram_tensor("y", [D, TOK], F32, kind="ExternalOutput").ap()
    with tile.TileContext(nc) as tc, ExitStack() as ctx:
        _body(ctx, tc, xT, wqkvT, woutT, expBT, y)
    nc.compile()
    _CACHE["nc"] = nc
    return nc


def kernel(x, w_qkv, w_out, bias_table, rel_idx):
    x = np.asarray(x, dtype=np.float32)
    w_qkv = np.asarray(w_qkv, dtype=np.float32)
    w_out = np.asarray(w_out, dtype=np.float32)
    bias_table = np.asarray(bias_table, dtype=np.float32)
    rel_idx = np.asarray(rel_idx)

    # host-side layout prep
    # x[0]: [l, X, Y, w1, w2, d] -> xT [d, (X Y l w1 w2)] bf16
    xt = np.ascontiguousarray(
        x[0].transpose(5, 1, 2, 0, 3, 4)).reshape(D, GX * GY * N).astype(ml_dtypes.bfloat16)
    wq = w_qkv.copy()
    if KFP8:
        # split the attention scale sqrt-evenly so q and k land mid-range in fp8
        s = SCALE ** 0.5
        wq[:D] *= s
        wq[D:2 * D] *= s
    else:
        wq[:D] *= SCALE                    # fold attention scale into Wq
    wqkvT = np.ascontiguousarray(wq.T).astype(ml_dtypes.bfloat16)
    woutT = np.ascontiguousarray(w_out.T).astype(ml_dtypes.bfloat16)
    bias = bias_table[rel_idx]             # [i, j, h]
    expBT = np.ascontiguousarray(
        np.exp(bias.transpose(1, 2, 0))).reshape(N, H * N).astype(ml_dtypes.bfloat16)

    nc = _build()
    in_maps = []
    for s in range(NCORES):
        xs = np.ascontiguousarray(xt[:, s * TOK:(s + 1) * TOK])
        in_maps.append({"xT": xs, "wqkvT": wqkvT, "woutT": woutT, "expBT": expBT})

    res = run_bass_kernel_spmd(nc, in_maps, core_ids=list(range(NCORES)), trace=TRACE)
    _CACHE["res"] = res
    if TRACE and res.exec_time_ns is not None:
        print(f"HW exec time: {res.exec_time_ns} ns")
        _CACHE["exec_time_ns"] = res.exec_time_ns

    # gather: per-core y [256, 9408] d-major -> [1, l, X, Y, w1, w2, d]
    out = np.empty((1, L, GX, GY, W, W, D), dtype=np.float32)
    for s in range(NCORES):
        yc = res.results[s]["y"].reshape(D, XPC, GY, L, W, W)
        out[0, :, s * XPC:(s + 1) * XPC] = yc.transpose(3, 1, 2, 4, 5, 0)
    return out


# revision 14
# speedup vs baseline: 1.2518x; 1.1597x over previous
"""Trainium2 Bass kernel for windowed 3D attention (sparse_attention).

Per window (256 windows on a 16x16 grid): N=294 tokens, d=256, 8 heads x 32.
qkv = x @ w_qkv.T ; A = softmax(q k^T/sqrt(dh) + bias) ; out = (A v) @ w_out.T
Sharding: data-parallel over the grid; core s takes X-rows [2s, 2s+2) = 32 windows.

v5 design (v2: ~469us, v3: ~429us):
  - bias applied as A = exp(S) * expB (expB host-precomputed, bf16 const);
    quad multiplies: jc0/jc1 on DVE (all-bf16 SBUF step-1 => 2x perf mode,
    ~770ns vs GPSIMD ~2400ns), jc2 on GPSIMD for queue balance.
  - S matmuls in FP8 (e4m3) with perf_mode=DoubleRow: Q/K stored as fp8
    [128, {q0,q1,k0,k1}, 2, 304] where pair-block 0 holds the data and the
    K-side pair-block 1 is zeroed (GPSIMD memset) — DoubleRow contracts
    sum_i W[:,i].T @ X[:,i], so zero K-odd blocks keep the math exact while
    the moving stream runs at 2 cols/cycle (S stream halves).  304 = 294
    padded so the pair-block stride is 16-aligned.  Host splits the softmax
    scale sqrt-evenly into Wq and Wk so both land mid-range in fp8.
  - rowsums via ones[jn,32] matmuls: row-sum REPLICATED across each head's
    32 partitions ([he, i] layout) -> reciprocal_approx_fast -> normalize O^T
    directly from PSUM on DVE.  Y^T = wout^T.T @ O_norm^T d-major.
  - S^T tiles [128, 2, 512] fp32 (2 banks, one head per bank).  HARD HW
    CONSTRAINT: a matmul PSUM output must start at a PSUM bank boundary
    (CoreSim does NOT model this).  Also: engine ops may not access >32
    partitions from a non-aligned partition start (BIR verifier).
  - S matmuls single-shot, K=32 row-packed 4-way via tile_position
    (32*(h%4), 0); RS/AV K=jn col-packed 4-way at (0, 32*hh).
  - PSUM: S-ring 3 x 2 banks + misc ring 2 x 1 bank = 8 banks.
  - 2-window QKV runway: prologue runs qkv(0)+qkv(1); in-loop fillers are
    qkv(w+2) + tail(w-1) interleaved with window w's 12 S/exp slots.
"""

import os
from contextlib import ExitStack

import numpy as np
import ml_dtypes

import concourse.bass as bass
import concourse.mybir as mybir
import concourse.tile as tile
from concourse import bacc
from concourse.bass_utils import run_bass_kernel_spmd

F32 = mybir.dt.float32
BF16 = mybir.dt.bfloat16
FP8 = mybir.dt.float8e4

L, W, D, H = 6, 7, 256, 8
DH = D // H                      # 32
N = L * W * W                    # 294
NP = 304                         # N padded to 16-elem multiple (DoubleRow stride)
GX = GY = 16
NCORES = 8
XPC = GX // NCORES               # X-rows per core
NW = int(os.environ.get("KNW", XPC * GY))   # 32 windows per core (overridable for debug)
TOK = NW * N                     # 9408 tokens per core
SCALE = DH ** -0.5
KFP8 = bool(int(os.environ.get("KFP8", "1")))   # fp8 DoubleRow S-stage

CH = [(0, 128), (128, 128), (256, 38)]    # j / i chunks


TRACE = False     # set by test.py for profiling runs
_CACHE = {}


def _body(ctx, tc, xT, wqkvT, woutT, expBT, y):
    nc = tc.nc

    const = ctx.enter_context(tc.tile_pool(name="const", bufs=1))
    xpool = ctx.enter_context(tc.tile_pool(name="xin", bufs=4))
    qkpool = ctx.enter_context(tc.tile_pool(name="qk", bufs=4))
    vpool = ctx.enter_context(tc.tile_pool(name="vtok", bufs=4))
    arawpool = ctx.enter_context(tc.tile_pool(name="araw", bufs=6))
    atpool = ctx.enter_context(tc.tile_pool(name="at", bufs=4))
    rrpool = ctx.enter_context(tc.tile_pool(name="rr", bufs=3))
    onpool = ctx.enter_context(tc.tile_pool(name="onorm", bufs=3))
    ypool = ctx.enter_context(tc.tile_pool(name="ysb", bufs=3))

    # PSUM: S-ring 3 x [128,2,512]f32 (2 banks each) + misc ring 2 x 1 bank
    ps_s = ctx.enter_context(tc.tile_pool(name="ps_s", bufs=3, space="PSUM"))
    ps_m = ctx.enter_context(tc.tile_pool(name="ps_m", bufs=2, space="PSUM"))

    # ---- resident constants ----
    # Declare all const tiles, but order the DMA queue so the first window's
    # dependencies (wqkv, then x0 — emitted by the caller right after
    # wqkv_dma()) land first; wv/wout follow, the 1.8MB expb goes last.
    wqkv_s = const.tile([128, 2, 2 * D], BF16)     # Q^T,K^T weight cols (pre-scaled)
    wv_s = const.tile([128, 2, D], BF16)
    wout_s = const.tile([128, 2, D], BF16)         # w_out^T [he, dout] he-chunked
    expb_s = const.tile([128, 3, H * N], BF16)     # exp(B^T)[j, (h,i)], j-chunked
    ones_b = const.tile([128, 32], BF16)
    nc.vector.memset(ones_b, 1.0)

    def wqkv_dma():
        nc.sync.dma_start(out=wqkv_s, in_=wqkvT.rearrange("(c p) n -> p c n", c=2)[:, :, 0:2 * D])

    def wtail_dma():
        nc.sync.dma_start(out=wv_s, in_=wqkvT.rearrange("(c p) n -> p c n", c=2)[:, :, 2 * D:3 * D])
        nc.sync.dma_start(out=wout_s, in_=woutT.rearrange("(c p) n -> p c n", c=2))

    def load_expb():
        # deferred: 1.8MB const queued AFTER the early x DMAs so the first
        # windows' QKV isn't stuck behind it on the DMA queue
        for jc, (j0, jn) in enumerate(CH):
            nc.sync.dma_start(out=expb_s[:jn, jc, :], in_=expBT[j0:j0 + jn, :])

    state = {}   # per-window live tiles

    def load_x(w):
        t0 = w * N
        xw = xpool.tile([128, 2, N], BF16, tag="xw", name=f"xw{w}")
        nc.sync.dma_start(out=xw, in_=xT.rearrange("(c p) t -> p c t", c=2)[:, :, t0:t0 + N])
        state[("x", w)] = xw

    def qkv_pieces(w):
        """7 closures: QKV projection of window w, one psum tile each."""
        xw = state[("x", w)]
        if KFP8:
            # fp8 DoubleRow layout: [m, pair, NP]; ALL pair-1 blocks zeroed —
            # uninitialized fp8 bytes can be NaN, and NaN*0 = NaN even against
            # the zeroed K side, so the Q-side junk must be cleared too.
            qk_sb = qkpool.tile([128, 4, 2, NP], FP8, tag="qk", name=f"qk{w}")
            nc.gpsimd.memset(qk_sb[:, :, 1, :], 0.0)
        else:
            qk_sb = qkpool.tile([128, 4, N], BF16, tag="qk", name=f"qk{w}")
        vtok = vpool.tile([128, 3, D], BF16, tag="vt", name=f"vt{w}")
        state[("qk", w)] = qk_sb
        state[("v", w)] = vtok

        def qk_piece(m):
            def run():
                pq = ps_m.tile([128, 512], F32, tag="ring", name=f"pq{w}_{m}")
                for kc in range(2):
                    nc.tensor.matmul(
                        pq[:, :N], wqkv_s[:, kc, m * 128:(m + 1) * 128], xw[:, kc, :],
                        start=(kc == 0), stop=(kc == 1))
                if KFP8:
                    nc.vector.tensor_copy(qk_sb[:, m, 0, :N], pq[:, :N])
                else:
                    nc.vector.tensor_copy(qk_sb[:, m, :], pq[:, :N])
            return run

        def v_piece(jc):
            def run():
                j0, jn = CH[jc]
                pv = ps_m.tile([128, 512], F32, tag="ring", name=f"pv{w}_{jc}")
                for kc in range(2):
                    nc.tensor.matmul(
                        pv[:jn, :D], xw[:, kc, j0:j0 + jn], wv_s[:, kc, :],
                        start=(kc == 0), stop=(kc == 1))
                nc.vector.tensor_copy(vtok[:jn, jc, :], pv[:jn, :D])
            return run

        return [qk_piece(m) for m in range(4)] + [v_piece(jc) for jc in range(3)]

    def tail_pieces(w):
        """6 closures: rowsums+recip / AV+normalize / out-proj of window w."""
        at = state[("at", w)]
        vtok = state[("v", w)]
        rr = rrpool.tile([128, 2, N], F32, tag="rr", name=f"rr{w}")
        onrm = onpool.tile([128, 2, N], BF16, tag="onrm", name=f"on{w}")
        ysb = ypool.tile([128, 2, N], F32, tag="ysb", name=f"ysb{w}")
        t0 = w * N

        def rs_piece(g):
            def run():
                prs = ps_m.tile([128, 512], F32, tag="ring", name=f"prs{w}_{g}")
                for jc, (j0, jn) in enumerate(CH):
                    for hh in range(4):
                        h = 4 * g + hh
                        nc.tensor.matmul(
                            prs[32 * hh:32 * hh + 32, :N],
                            ones_b[:jn, :], at[:jn, jc, h, :],
                            start=(jc == 0), stop=(jc == 2),
                            tile_position=(0, 32 * hh), skip_group_check=True)
                nc.vector.reciprocal_approx_fast(out=rr[:, g, :], in_=prs[:, :N])
            return run

        def av_piece(g):
            def run():
                po = ps_m.tile([128, 512], F32, tag="ring", name=f"po{w}_{g}")
                for jc, (j0, jn) in enumerate(CH):
                    for hh in range(4):
                        h = 4 * g + hh
                        nc.tensor.matmul(
                            po[32 * hh:32 * hh + 32, :N],
                            vtok[:jn, jc, 32 * h:32 * h + 32], at[:jn, jc, h, :],
                            start=(jc == 0), stop=(jc == 2),
                            tile_position=(0, 32 * hh), skip_group_check=True)
                nc.vector.tensor_tensor(
                    out=onrm[:, g, :], in0=po[:, :N], in1=rr[:, g, :],
                    op=mybir.AluOpType.mult)
            return run

        def y_piece(d):
            def run():
                py = ps_m.tile([128, 512], F32, tag="ring", name=f"py{w}_{d}")
                for g in range(2):
                    nc.tensor.matmul(
                        py[:, :N], wout_s[:, g, 128 * d:128 * (d + 1)], onrm[:, g, :],
                        start=(g == 0), stop=(g == 1))
                nc.vector.tensor_copy(ysb[:, d, :], py[:, :N])
                nc.sync.dma_start(out=y[128 * d:128 * (d + 1), t0:t0 + N], in_=ysb[:, d, :])
            return run

        return [rs_piece(0), rs_piece(1), av_piece(0), av_piece(1),
                y_piece(0), y_piece(1)]

    def s_slot(w, jc, g2, qd_tiles):
        """S matmuls + exp for heads (2*g2, 2*g2+1) of (window w, j-chunk jc),
        then (after the second tile of a quad) the bias multiply."""
        qk_sb = state[("qk", w)]
        at = state[("at", w)]
        j0, jn = CH[jc]
        s_t = ps_s.tile([128, 2, 512], F32, tag="sring", name=f"s{w}_{jc}_{g2}")
        for p in range(2):
            h = 2 * g2 + p
            hh = h % 4
            g = h // 4
            if KFP8:
                nc.tensor.matmul(
                    s_t[:jn, p, :N],
                    qk_sb[32 * hh:32 * hh + 32, 2 + g, :, j0:j0 + jn],  # K^T [32,2,jn]
                    qk_sb[32 * hh:32 * hh + 32, g, :, :N],             # Q^T [32,2,294]
                    start=True, stop=True, skip_group_check=True,
                    tile_position=(32 * hh, 0),
                    perf_mode=mybir.MatmulPerfMode.DoubleRow)
            else:
                nc.tensor.matmul(
                    s_t[:jn, p, :N],
                    qk_sb[32 * hh:32 * hh + 32, 2 + g, j0:j0 + jn],    # K^T [32, jn]
                    qk_sb[32 * hh:32 * hh + 32, g, :],                 # Q^T [32, 294]
                    start=True, stop=True, skip_group_check=True,
                    tile_position=(32 * hh, 0))
        # quad = two consecutive 2-head tiles share one araw tile so the
        # bias multiply runs as one [jn, 4, 294] op
        qd = g2 // 2
        sub = g2 % 2
        if sub == 0:
            qd_tiles[qd] = arawpool.tile(
                [128, 4, N], BF16, tag="araw", name=f"ar{w}_{jc}_{qd}")
        araw = qd_tiles[qd]
        nc.scalar.activation(
            araw[:jn, 2 * sub:2 * sub + 2, :], s_t[:jn, :, :N],
            mybir.ActivationFunctionType.Exp)
        if sub == 1:
            # bias multiply for the whole quad (heads 4*qd .. 4*qd+4).
            # jc0/jc1 on DVE (2x perf mode ~770ns); jc2 on GPSIMD (~2400ns)
            # to keep the DVE queue short.
            eng = nc.vector if jc < 2 else nc.gpsimd
            eng.tensor_tensor(
                out=at[:jn, jc, 4 * qd:4 * qd + 4, :],
                in0=araw[:jn, :, :],
                in1=expb_s[:jn, jc, 4 * qd * N:(4 * qd + 4) * N].rearrange(
                    "p (h i) -> p h i", h=4),
                op=mybir.AluOpType.mult)

    # ---- prologue: 2-window QKV runway so in-loop fillers are w+2 ----
    wqkv_dma()
    load_x(0)
    load_x(1)
    wtail_dma()
    load_x(2)
    load_expb()
    load_x(3)
    for f in qkv_pieces(0):
        f()
    for f in qkv_pieces(1):
        f()

    # ---- pipelined window loop ----
    for w in range(NW):
        if w + 4 < NW:
            load_x(w + 4)
        state[("at", w)] = atpool.tile([128, 3, H, N], BF16, tag="at", name=f"at{w}")

        # qkv(w+2) first: tail(w-1) pieces early in the window stall the
        # in-order PE queue on at(w-1) readiness (measured +110us when
        # tail-first).
        fillers = []
        if w + 2 < NW:
            fillers += qkv_pieces(w + 2)
        if w > 0:
            fillers += tail_pieces(w - 1)

        # interleave ~1 filler piece per S slot to keep PE fed without
        # delaying the next S tile behind a long PE burst
        nslots = 12
        total = len(fillers)
        taken = 0
        # (measured: jc2-first slot order 490us, jc-natural 417us)
        for si, (jc, g2) in enumerate([(j, g) for j in range(3) for g in range(4)]):
            if g2 == 0:
                qd_tiles = {}
            s_slot(w, jc, g2, qd_tiles)
            want = total * (si + 1) // nslots
            while taken < want:
                fillers[taken]()
                taken += 1
        while taken < total:
            fillers[taken]()
            taken += 1

    # final tail
    for f in tail_pieces(NW - 1):
        f()


def _build():
    if "nc" in _CACHE:
        return _CACHE["nc"]
    nc = bacc.Bacc("TRN2", target_bir_lowering=False)
    xT = nc.dram_tensor("xT", [D, TOK], BF16, kind="ExternalInput").ap()
    wqkvT = nc.dram_tensor("wqkvT", [D, 3 * D], BF16, kind="ExternalInput").ap()
    woutT = nc.dram_tensor("woutT", [D, D], BF16, kind="ExternalInput").ap()
    expBT = nc.dram_tensor("expBT", [N, H * N], BF16, kind="ExternalInput").ap()
    y = nc.dram_tensor("y", [D, TOK], F32, kind="ExternalOutput").ap()
    with tile.TileContext(nc) as tc, ExitStack() as ctx:
        _body(ctx, tc, xT, wqkvT, woutT, expBT, y)
    nc.compile()
    _CACHE["nc"] = nc
    return nc


def kernel(x, w_qkv, w_out, bias_table, rel_idx):
    x = np.asarray(x, dtype=np.float32)
    w_qkv = np.asarray(w_qkv, dtype=np.float32)
    w_out = np.asarray(w_out, dtype=np.float32)
    bias_table = np.asarray(bias_table, dtype=np.float32)
    rel_idx = np.asarray(rel_idx)

    # host-side layout prep
    # x[0]: [l, X, Y, w1, w2, d] -> xT [d, (X Y l w1 w2)] bf16
    xt = np.ascontiguousarray(
        x[0].transpose(5, 1, 2, 0, 3, 4)).reshape(D, GX * GY * N).astype(ml_dtypes.bfloat16)
    wq = w_qkv.copy()
    if KFP8:
        # split the attention scale sqrt-evenly so q and k land mid-range in fp8
        s = SCALE ** 0.5
        wq[:D] *= s
        wq[D:2 * D] *= s
    else:
        wq[:D] *= SCALE                    # fold attention scale into Wq
    wqkvT = np.ascontiguousarray(wq.T).astype(ml_dtypes.bfloat16)
    woutT = np.ascontiguousarray(w_out.T).astype(ml_dtypes.bfloat16)
    bias = bias_table[rel_idx]             # [i, j, h]
    expBT = np.ascontiguousarray(
        np.exp(bias.transpose(1, 2, 0))).reshape(N, H * N).astype(ml_dtypes.bfloat16)

    nc = _build()
    in_maps = []
    for s in range(NCORES):
        xs = np.ascontiguousarray(xt[:, s * TOK:(s + 1) * TOK])
        in_maps.append({"xT": xs, "wqkvT": wqkvT, "woutT": woutT, "expBT": expBT})

    res = run_bass_kernel_spmd(nc, in_maps, core_ids=list(range(NCORES)), trace=TRACE)
    _CACHE["res"] = res
    if TRACE and res.exec_time_ns is not None:
        print(f"HW exec time: {res.exec_time_ns} ns")
        _CACHE["exec_time_ns"] = res.exec_time_ns

    # gather: per-core y [256, 9408] d-major -> [1, l, X, Y, w1, w2, d]
    out = np.empty((1, L, GX, GY, W, W, D), dtype=np.float32)
    for s in range(NCORES):
        yc = res.results[s]["y"].reshape(D, XPC, GY, L, W, W)
        out[0, :, s * XPC:(s + 1) * XPC] = yc.transpose(3, 1, 2, 4, 5, 0)
    return out


# revision 18
# speedup vs baseline: 1.2589x; 1.0057x over previous
"""Trainium2 Bass kernel for windowed 3D attention (sparse_attention).

Per window (256 windows on a 16x16 grid): N=294 tokens, d=256, 8 heads x 32.
qkv = x @ w_qkv.T ; A = softmax(q k^T/sqrt(dh) + bias) ; out = (A v) @ w_out.T
Sharding: data-parallel over the grid; core s takes X-rows [2s, 2s+2) = 32 windows.

v5 design (v2: ~469us, v3: ~429us):
  - bias applied as A = exp(S) * expB (expB host-precomputed, bf16 const);
    quad multiplies: jc0/jc1 on DVE (all-bf16 SBUF step-1 => 2x perf mode,
    ~770ns vs GPSIMD ~2400ns), jc2 on GPSIMD for queue balance.
  - S matmuls in FP8 (e4m3) with perf_mode=DoubleRow: Q/K stored as fp8
    [128, {q0,q1,k0,k1}, 2, 304] where pair-block 0 holds the data and the
    K-side pair-block 1 is zeroed (GPSIMD memset) — DoubleRow contracts
    sum_i W[:,i].T @ X[:,i], so zero K-odd blocks keep the math exact while
    the moving stream runs at 2 cols/cycle (S stream halves).  304 = 294
    padded so the pair-block stride is 16-aligned.  Host splits the softmax
    scale sqrt-evenly into Wq and Wk so both land mid-range in fp8.
  - rowsums via ones[jn,32] matmuls: row-sum REPLICATED across each head's
    32 partitions ([he, i] layout) -> reciprocal_approx_fast -> normalize O^T
    directly from PSUM on DVE.  Y^T = wout^T.T @ O_norm^T d-major.
  - S^T tiles [128, 2, 512] fp32 (2 banks, one head per bank).  HARD HW
    CONSTRAINT: a matmul PSUM output must start at a PSUM bank boundary
    (CoreSim does NOT model this).  Also: engine ops may not access >32
    partitions from a non-aligned partition start (BIR verifier).
  - S matmuls single-shot, K=32 row-packed 4-way via tile_position
    (32*(h%4), 0); RS/AV K=jn col-packed 4-way at (0, 32*hh).
  - PSUM: S-ring 3 x 2 banks + misc ring 2 x 1 bank = 8 banks.
  - 2-window QKV runway: prologue runs qkv(0)+qkv(1); in-loop fillers are
    qkv(w+2) + tail(w-1) interleaved with window w's 12 S/exp slots.
"""

import os
from contextlib import ExitStack

import numpy as np
import ml_dtypes

import concourse.bass as bass
import concourse.mybir as mybir
import concourse.tile as tile
from concourse import bacc
from concourse.bass_utils import run_bass_kernel_spmd

F32 = mybir.dt.float32
BF16 = mybir.dt.bfloat16
FP8 = mybir.dt.float8e4

L, W, D, H = 6, 7, 256, 8
DH = D // H                      # 32
N = L * W * W                    # 294
NP = 304                         # N padded to 16-elem multiple (DoubleRow stride)
GX = GY = 16
NCORES = 8
XPC = GX // NCORES               # X-rows per core
NW = int(os.environ.get("KNW", XPC * GY))   # 32 windows per core (overridable for debug)
TOK = NW * N                     # 9408 tokens per core
SCALE = DH ** -0.5
KFP8 = bool(int(os.environ.get("KFP8", "1")))   # fp8 DoubleRow S-stage

CH = [(0, 128), (128, 128), (256, 38)]    # j / i chunks


TRACE = False     # set by test.py for profiling runs
_CACHE = {}


def _body(ctx, tc, xT, wqkvT, woutT, expBT, y):
    nc = tc.nc

    const = ctx.enter_context(tc.tile_pool(name="const", bufs=1))
    xpool = ctx.enter_context(tc.tile_pool(name="xin", bufs=4))
    qkpool = ctx.enter_context(tc.tile_pool(name="qk", bufs=4))
    vpool = ctx.enter_context(tc.tile_pool(name="vtok", bufs=4))
    arawpool = ctx.enter_context(tc.tile_pool(name="araw", bufs=6))
    atpool = ctx.enter_context(tc.tile_pool(name="at", bufs=4))
    rrpool = ctx.enter_context(tc.tile_pool(name="rr", bufs=3))
    onpool = ctx.enter_context(tc.tile_pool(name="onorm", bufs=3))
    ypool = ctx.enter_context(tc.tile_pool(name="ysb", bufs=3))

    # PSUM: S-ring 3 x [128,2,512]f32 (2 banks each) + misc ring 2 x 1 bank
    ps_s = ctx.enter_context(tc.tile_pool(name="ps_s", bufs=3, space="PSUM"))
    ps_m = ctx.enter_context(tc.tile_pool(name="ps_m", bufs=2, space="PSUM"))

    # ---- resident constants ----
    # Declare all const tiles, but order the DMA queue so the first window's
    # dependencies (wqkv, then x0 — emitted by the caller right after
    # wqkv_dma()) land first; wv/wout follow, the 1.8MB expb goes last.
    wqkv_s = const.tile([128, 2, 2 * D], BF16)     # Q^T,K^T weight cols (pre-scaled)
    wv_s = const.tile([128, 2, D], BF16)
    wout_s = const.tile([128, 2, D], BF16)         # w_out^T [he, dout] he-chunked
    expb_s = const.tile([128, 3, H * N], BF16)     # exp(B^T)[j, (h,i)], j-chunked
    ones_b = const.tile([128, 32], BF16)
    nc.vector.memset(ones_b, 1.0)

    def wqkv_dma():
        nc.sync.dma_start(out=wqkv_s, in_=wqkvT.rearrange("(c p) n -> p c n", c=2)[:, :, 0:2 * D])

    def wtail_dma():
        nc.sync.dma_start(out=wv_s, in_=wqkvT.rearrange("(c p) n -> p c n", c=2)[:, :, 2 * D:3 * D])
        nc.sync.dma_start(out=wout_s, in_=woutT.rearrange("(c p) n -> p c n", c=2))

    def load_expb():
        # deferred: 1.8MB const queued AFTER the early x DMAs so the first
        # windows' QKV isn't stuck behind it on the DMA queue
        for jc, (j0, jn) in enumerate(CH):
            nc.sync.dma_start(out=expb_s[:jn, jc, :], in_=expBT[j0:j0 + jn, :])

    state = {}   # per-window live tiles

    def load_x(w):
        t0 = w * N
        xw = xpool.tile([128, 2, N], BF16, tag="xw", name=f"xw{w}")
        nc.sync.dma_start(out=xw, in_=xT.rearrange("(c p) t -> p c t", c=2)[:, :, t0:t0 + N])
        state[("x", w)] = xw

    def qkv_pieces(w):
        """7 closures: QKV projection of window w, one psum tile each."""
        xw = state[("x", w)]
        if KFP8:
            # fp8 DoubleRow layout: [m, pair, NP]; ALL pair-1 blocks zeroed —
            # uninitialized fp8 bytes can be NaN, and NaN*0 = NaN even against
            # the zeroed K side, so the Q-side junk must be cleared too.
            qk_sb = qkpool.tile([128, 4, 2, NP], FP8, tag="qk", name=f"qk{w}")
            nc.gpsimd.memset(qk_sb[:, :, 1, :], 0.0)
        else:
            qk_sb = qkpool.tile([128, 4, N], BF16, tag="qk", name=f"qk{w}")
        vtok = vpool.tile([128, 3, D], BF16, tag="vt", name=f"vt{w}")
        state[("qk", w)] = qk_sb
        state[("v", w)] = vtok

        def qk_piece(m):
            def run():
                pq = ps_m.tile([128, 512], F32, tag="ring", name=f"pq{w}_{m}")
                for kc in range(2):
                    nc.tensor.matmul(
                        pq[:, :N], wqkv_s[:, kc, m * 128:(m + 1) * 128], xw[:, kc, :],
                        start=(kc == 0), stop=(kc == 1))
                if KFP8:
                    nc.vector.tensor_copy(qk_sb[:, m, 0, :N], pq[:, :N])
                else:
                    nc.vector.tensor_copy(qk_sb[:, m, :], pq[:, :N])
            return run

        def v_piece(jc):
            def run():
                j0, jn = CH[jc]
                pv = ps_m.tile([128, 512], F32, tag="ring", name=f"pv{w}_{jc}")
                for kc in range(2):
                    nc.tensor.matmul(
                        pv[:jn, :D], xw[:, kc, j0:j0 + jn], wv_s[:, kc, :],
                        start=(kc == 0), stop=(kc == 1))
                nc.vector.tensor_copy(vtok[:jn, jc, :], pv[:jn, :D])
            return run

        return [qk_piece(m) for m in range(4)] + [v_piece(jc) for jc in range(3)]

    def tail_pieces(w):
        """6 closures: rowsums+recip / AV+normalize / out-proj of window w."""
        at = state[("at", w)]
        vtok = state[("v", w)]
        rr = rrpool.tile([128, 2, N], F32, tag="rr", name=f"rr{w}")
        onrm = onpool.tile([128, 2, N], BF16, tag="onrm", name=f"on{w}")
        ysb = ypool.tile([128, 2, N], F32, tag="ysb", name=f"ysb{w}")
        t0 = w * N

        def rs_piece(g):
            def run():
                prs = ps_m.tile([128, 512], F32, tag="ring", name=f"prs{w}_{g}")
                for jc, (j0, jn) in enumerate(CH):
                    for hh in range(4):
                        h = 4 * g + hh
                        nc.tensor.matmul(
                            prs[32 * hh:32 * hh + 32, :N],
                            ones_b[:jn, :], at[:jn, jc, h, :],
                            start=(jc == 0), stop=(jc == 2),
                            tile_position=(0, 32 * hh), skip_group_check=True)
                nc.vector.reciprocal_approx_fast(out=rr[:, g, :], in_=prs[:, :N])
            return run

        def av_piece(g):
            def run():
                po = ps_m.tile([128, 512], F32, tag="ring", name=f"po{w}_{g}")
                for jc, (j0, jn) in enumerate(CH):
                    for hh in range(4):
                        h = 4 * g + hh
                        nc.tensor.matmul(
                            po[32 * hh:32 * hh + 32, :N],
                            vtok[:jn, jc, 32 * h:32 * h + 32], at[:jn, jc, h, :],
                            start=(jc == 0), stop=(jc == 2),
                            tile_position=(0, 32 * hh), skip_group_check=True)
                nc.vector.tensor_tensor(
                    out=onrm[:, g, :], in0=po[:, :N], in1=rr[:, g, :],
                    op=mybir.AluOpType.mult)
            return run

        def y_piece(d):
            def run():
                py = ps_m.tile([128, 512], F32, tag="ring", name=f"py{w}_{d}")
                for g in range(2):
                    nc.tensor.matmul(
                        py[:, :N], wout_s[:, g, 128 * d:128 * (d + 1)], onrm[:, g, :],
                        start=(g == 0), stop=(g == 1))
                nc.vector.tensor_copy(ysb[:, d, :], py[:, :N])
                nc.sync.dma_start(out=y[128 * d:128 * (d + 1), t0:t0 + N], in_=ysb[:, d, :])
            return run

        return [rs_piece(0), rs_piece(1), av_piece(0), av_piece(1),
                y_piece(0), y_piece(1)]

    def s_slot(w, jc, g2, qd_tiles):
        """S matmuls + exp for heads (2*g2, 2*g2+1) of (window w, j-chunk jc),
        then (after the second tile of a quad) the bias multiply."""
        qk_sb = state[("qk", w)]
        at = state[("at", w)]
        j0, jn = CH[jc]
        s_t = ps_s.tile([128, 2, 512], F32, tag="sring", name=f"s{w}_{jc}_{g2}")
        for p in range(2):
            h = 2 * g2 + p
            hh = h % 4
            g = h // 4
            if KFP8:
                nc.tensor.matmul(
                    s_t[:jn, p, :N],
                    qk_sb[32 * hh:32 * hh + 32, 2 + g, :, j0:j0 + jn],  # K^T [32,2,jn]
                    qk_sb[32 * hh:32 * hh + 32, g, :, :N],             # Q^T [32,2,294]
                    start=True, stop=True, skip_group_check=True,
                    tile_position=(32 * hh, 0),
                    perf_mode=mybir.MatmulPerfMode.DoubleRow)
            else:
                nc.tensor.matmul(
                    s_t[:jn, p, :N],
                    qk_sb[32 * hh:32 * hh + 32, 2 + g, j0:j0 + jn],    # K^T [32, jn]
                    qk_sb[32 * hh:32 * hh + 32, g, :],                 # Q^T [32, 294]
                    start=True, stop=True, skip_group_check=True,
                    tile_position=(32 * hh, 0))
        # quad = two consecutive 2-head tiles share one araw tile so the
        # bias multiply runs as one [jn, 4, 294] op
        qd = g2 // 2
        sub = g2 % 2
        if sub == 0:
            qd_tiles[qd] = arawpool.tile(
                [128, 4, N], BF16, tag="araw", name=f"ar{w}_{jc}_{qd}")
        araw = qd_tiles[qd]
        nc.scalar.activation(
            araw[:jn, 2 * sub:2 * sub + 2, :], s_t[:jn, :, :N],
            mybir.ActivationFunctionType.Exp)
        if sub == 1:
            # bias multiply for the whole quad (heads 4*qd .. 4*qd+4).
            # jc0/jc1 on DVE (2x perf mode ~770ns); jc2 on GPSIMD (~2400ns)
            # to keep the DVE queue short — except in the LAST window, where
            # the 2.4us gpsimd mult would sit on the final tail's critical
            # path (measured 1.4us PE stall in the epilogue).
            eng = nc.vector if (jc < 2 or w == NW - 1) else nc.gpsimd
            eng.tensor_tensor(
                out=at[:jn, jc, 4 * qd:4 * qd + 4, :],
                in0=araw[:jn, :, :],
                in1=expb_s[:jn, jc, 4 * qd * N:(4 * qd + 4) * N].rearrange(
                    "p (h i) -> p h i", h=4),
                op=mybir.AluOpType.mult)

    # ---- prologue: qkv(0) only; qkv(1)+qkv(2) run as window-0 fillers so
    # the first S slots start as soon as qk(0) is cast (~2.5us) ----
    wqkv_dma()
    load_x(0)
    load_x(1)
    wtail_dma()
    load_x(2)
    load_expb()
    load_x(3)
    for f in qkv_pieces(0):
        f()

    # ---- pipelined window loop ----
    for w in range(NW):
        if w + 4 < NW:
            load_x(w + 4)
        state[("at", w)] = atpool.tile([128, 3, H, N], BF16, tag="at", name=f"at{w}")

        # qkv(w+2) first: tail(w-1) pieces early in the window stall the
        # in-order PE queue on at(w-1) readiness (measured +110us when
        # tail-first).
        fillers = []
        if w == 0:
            fillers += qkv_pieces(1) + qkv_pieces(2)
        elif w + 2 < NW:
            fillers += qkv_pieces(w + 2)
        if w > 0:
            fillers += tail_pieces(w - 1)

        # interleave ~1 filler piece per S slot to keep PE fed without
        # delaying the next S tile behind a long PE burst
        nslots = 12
        total = len(fillers)
        taken = 0
        # (measured: jc2-first slot order for ALL windows regressed; only the
        # last window uses it, so its at[jc2] quads finish ~8 slots before
        # the final serial tail consumes them)
        jcs = (2, 0, 1) if w == NW - 1 else (0, 1, 2)
        for si, (jc, g2) in enumerate([(j, g) for j in jcs for g in range(4)]):
            if g2 == 0:
                qd_tiles = {}
            s_slot(w, jc, g2, qd_tiles)
            want = total * (si + 1) // nslots
            while taken < want:
                fillers[taken]()
                taken += 1
        while taken < total:
            fillers[taken]()
            taken += 1

    # final tail
    for f in tail_pieces(NW - 1):
        f()


def _build():
    if "nc" in _CACHE:
        return _CACHE["nc"]
    nc = bacc.Bacc("TRN2", target_bir_lowering=False)
    xT = nc.dram_tensor("xT", [D, TOK], BF16, kind="ExternalInput").ap()
    wqkvT = nc.dram_tensor("wqkvT", [D, 3 * D], BF16, kind="ExternalInput").ap()
    woutT = nc.dram_tensor("woutT", [D, D], BF16, kind="ExternalInput").ap()
    expBT = nc.dram_tensor("expBT", [N, H * N], BF16, kind="ExternalInput").ap()
    y = nc.dram_tensor("y", [D, TOK], F32, kind="ExternalOutput").ap()
    with tile.TileContext(nc) as tc, ExitStack() as ctx:
        _body(ctx, tc, xT, wqkvT, woutT, expBT, y)
    nc.compile()
    _CACHE["nc"] = nc
    return nc


def kernel(x, w_qkv, w_out, bias_table, rel_idx):
    x = np.asarray(x, dtype=np.float32)
    w_qkv = np.asarray(w_qkv, dtype=np.float32)
    w_out = np.asarray(w_out, dtype=np.float32)
    bias_table = np.asarray(bias_table, dtype=np.float32)
    rel_idx = np.asarray(rel_idx)

    # host-side layout prep
    # x[0]: [l, X, Y, w1, w2, d] -> xT [d, (X Y l w1 w2)] bf16
    xt = np.ascontiguousarray(
        x[0].transpose(5, 1, 2, 0, 3, 4)).reshape(D, GX * GY * N).astype(ml_dtypes.bfloat16)
    wq = w_qkv.copy()
    if KFP8:
        # split the attention scale sqrt-evenly so q and k land mid-range in fp8
        s = SCALE ** 0.5
        wq[:D] *= s
        wq[D:2 * D] *= s
    else:
        wq[:D] *= SCALE                    # fold attention scale into Wq
    wqkvT = np.ascontiguousarray(wq.T).astype(ml_dtypes.bfloat16)
    woutT = np.ascontiguousarray(w_out.T).astype(ml_dtypes.bfloat16)
    bias = bias_table[rel_idx]             # [i, j, h]
    expBT = np.ascontiguousarray(
        np.exp(bias.transpose(1, 2, 0))).reshape(N, H * N).astype(ml_dtypes.bfloat16)

    nc = _build()
    in_maps = []
    for s in range(NCORES):
        xs = np.ascontiguousarray(xt[:, s * TOK:(s + 1) * TOK])
        in_maps.append({"xT": xs, "wqkvT": wqkvT, "woutT": woutT, "expBT": expBT})

    res = run_bass_kernel_spmd(nc, in_maps, core_ids=list(range(NCORES)), trace=TRACE)
    _CACHE["res"] = res
    if TRACE and res.exec_time_ns is not None:
        print(f"HW exec time: {res.exec_time_ns} ns")
        _CACHE["exec_time_ns"] = res.exec_time_ns

    # gather: per-core y [256, 9408] d-major -> [1, l, X, Y, w1, w2, d]
    out = np.empty((1, L, GX, GY, W, W, D), dtype=np.float32)
    for s in range(NCORES):
        yc = res.results[s]["y"].reshape(D, XPC, GY, L, W, W)
        out[0, :, s * XPC:(s + 1) * XPC] = yc.transpose(3, 1, 2, 4, 5, 0)
    return out
